# revision 7
# baseline (speedup 1.0000x reference)
"""Trainium2 Bass kernel for nn_EncoderOnlyBlock (4-head full-dim encoder block).

Sharding: fully data-parallel, no collectives. 8 cores = (batch b, seq-half).
Each core computes its 1024 query tokens end-to-end for all 4 heads; K work
for the full 2048-token batch row is recomputed on both cores of a batch
(the only duplicated work).

All heavy matmuls run in fp8-e4m3 DoubleRow mode (2 k-blocks per instruction,
2x bf16 throughput), with power-of-2 scale factors folded into the operands
so PSUM results are rescaled for free on the copy-out:
  x -> fp8 x*16;  Wq/Wk/Wv -> fp8 *4096;  W1 -> fp8 *512
  Q^T_h = Wq_h^T x^T (+16*bq via activation bias), stored fp8 = 16*Q
  K^T_h = Wk_h^T x^T stored fp8 = 16*K   (bk dropped: softmax invariant)
  S psum = 256*S;  A = exp(S/sqrtD) via activation scale 1/8192, bf16,
  normalized by 1/rowsum; A^T stored fp8 = 128*A (scaled in transpose copy)
  M_h = x^T A^T (psum 2048*M) stored fp8 = 32*M   (A@V == Wv^T@M reassoc,
  bv_h folded into cvec since rows of A sum to 1)
  hd^T_h = Wv_h^T M (psum 2^17*hd) stored fp8 = 16*hd
  proj = sum_h hd_h @ W1_h (psum 8192*proj) accumulated bf16 unscaled
  cvec = b1 + sum_h bv_h @ W1_h is folded into xres on the host.
  u1 = xres + proj;  yhat = LN1(u1);  u2 = y + yhat@W2' + bu  (bf16 chain)
  out = LN2(u2)
LN means/vars via sum & sum-of-squares accumulators (E[x^2]-mu^2); g1/be1 and
g2/be2 application is skipped when they are exactly ones/zeros (checked on
host; g1/be1 additionally fold into W2'/bu which is exact in that case).
"""

import numpy as np
import ml_dtypes

BF = ml_dtypes.bfloat16
F8 = ml_dtypes.float8_e4m3
P = 128
D = 1024
S = 2048
SI = 1024
H = 4
ET = D // P       # 8 e/d/f 128-blocks
SJT = S // P      # 16 sj 128-blocks
SIT = SI // P     # 8 si 128-blocks
SCALE = 1.0 / 32.0  # 1/sqrt(D)
EPS = 1e-5

SX = 16.0         # x fp8 scale
SW = 4096.0       # Wq/Wk/Wv fp8 scale
SW1 = 512.0       # W1 fp8 scale
SQK = 16.0        # Q/K fp8 storage scale
SA = 128.0        # A^T fp8 storage scale
SM = 32.0         # M fp8 storage scale
SH = 16.0         # head^T fp8 storage scale

_CACHE = {}


def _emit(nc, tc, A, trivial_gbe):
    """Emit the per-core program. A: dict name -> dram AP."""
    from contextlib import ExitStack

    import concourse.bass as bass
    import concourse.mybir as mybir
    from concourse.masks import make_identity

    f32 = mybir.dt.float32
    bf16 = mybir.dt.bfloat16
    fp8 = mybir.dt.float8e4
    Act = mybir.ActivationFunctionType
    Alu = mybir.AluOpType
    DR = mybir.MatmulPerfMode.DoubleRow

    with ExitStack() as ctx:
        consts = ctx.enter_context(tc.tile_pool(name="consts", bufs=1))
        psA = ctx.enter_context(tc.tile_pool(name="psA", bufs=3, space="PSUM"))
        psB = ctx.enter_context(tc.tile_pool(name="psB", bufs=2, space="PSUM"))

        ident = consts.tile([P, P], bf16, tag="ident")
        make_identity(nc, ident[:])
        bqr_sb = consts.tile([P, H * ET], f32, tag="bqr")
        nc.sync.dma_start(out=bqr_sb[:], in_=A["bqr"][:])
        buv_sb = consts.tile([1, D], bf16, tag="buv")
        nc.sync.dma_start(out=buv_sb[:], in_=A["buv"][:])
        ones_sb = consts.tile([1, P], bf16, tag="ones")
        nc.vector.memset(ones_sb[:], 1.0)
        eps_sb = consts.tile([P, 1], f32, tag="eps")
        nc.vector.memset(eps_sb[:], EPS)

        # attention-side pools close after the last m-chain; mid pools after the
        # last WvM; tail pools live through the interleaved W1(h3)+LN loop.
        tail_ctx = ExitStack()
        w1_pool = tail_ctx.enter_context(tc.tile_pool(name="w1", bufs=2))
        ht_pool = tail_ctx.enter_context(tc.tile_pool(name="ht", bufs=1))
        proj_pool = tail_ctx.enter_context(tc.tile_pool(name="pj", bufs=1))
        mid_ctx = ExitStack()
        wqkv_pool = mid_ctx.enter_context(tc.tile_pool(name="wqkv", bufs=3))
        m_pool = mid_ctx.enter_context(tc.tile_pool(name="m", bufs=1))
        attn_ctx = ExitStack()
        xpool = attn_ctx.enter_context(tc.tile_pool(name="xp", bufs=1))
        qt_pool = attn_ctx.enter_context(tc.tile_pool(name="qt", bufs=1))
        kt_pool = attn_ctx.enter_context(tc.tile_pool(name="kt", bufs=1))
        attn_pool = attn_ctx.enter_context(tc.tile_pool(name="at", bufs=3))
        atT_pool = attn_ctx.enter_context(tc.tile_pool(name="atT", bufs=1))
        red_pool = attn_ctx.enter_context(tc.tile_pool(name="red", bufs=8))

        # x^T tiles first (first K-chain consumes them; low halves first so the
        # hs=0 chains can start), x natural after head-0's weights are queued
        # (not needed until the M phase of head 0)
        xt_sb = xpool.tile([P, ET, S], fp8, tag="xt")
        for hs in range(2):
            for c in range(ET):
                nc.sync.dma_start(
                    out=xt_sb[:, c, hs * 1024:(hs + 1) * 1024],
                    in_=A["xt"][c * P:(c + 1) * P, hs * 1024:(hs + 1) * 1024],
                )
        xn_sb = xpool.tile([P, SJT, D], fp8, tag="xn")

        proj_sb = proj_pool.tile([P, SIT, D], bf16, tag="proj")

        for h in range(H):
            # ---- K^T = Wk^T @ x^T : [e, sj], fp8 DoubleRow over kc pairs
            kt_sb = kt_pool.tile([P, ET, S], fp8, tag="kt")
            for c in range(ET):
                wk_c = wqkv_pool.tile([P, ET, P], fp8, tag="wqkv")
                nc.sync.dma_start(out=wk_c[:], in_=A["wkb"][h, c])
                for hs in range(2):
                    ps = psA.tile([P, 1024], f32, tag="psA")
                    for nb in range(2):
                        for kp in range(ET // 2):
                            nc.tensor.matmul(
                                ps[:, nb * 512:(nb + 1) * 512],
                                lhsT=wk_c[:, 2 * kp:2 * kp + 2, :],
                                rhs=xt_sb[:, 2 * kp:2 * kp + 2,
                                          hs * 1024 + nb * 512:hs * 1024 + (nb + 1) * 512],
                                start=(kp == 0), stop=(kp == ET // 2 - 1),
                                perf_mode=DR,
                            )
                    nc.scalar.mul(kt_sb[:, c, hs * 1024:(hs + 1) * 1024], ps[:],
                                  SQK / (SW * SX))

            # ---- Q^T = Wq^T @ x^T[:, :1024] + bq : [e, si]
            qt_sb = qt_pool.tile([P, ET, SI], fp8, tag="qt")
            for c in range(ET):
                wq_c = wqkv_pool.tile([P, ET, P], fp8, tag="wqkv")
                nc.sync.dma_start(out=wq_c[:], in_=A["wqb"][h, c])
                ps = psA.tile([P, 1024], f32, tag="psA")
                for nb in range(2):
                    for kp in range(ET // 2):
                        nc.tensor.matmul(
                            ps[:, nb * 512:(nb + 1) * 512],
                            lhsT=wq_c[:, 2 * kp:2 * kp + 2, :],
                            rhs=xt_sb[:, 2 * kp:2 * kp + 2, nb * 512:(nb + 1) * 512],
                            start=(kp == 0), stop=(kp == ET // 2 - 1),
                            perf_mode=DR,
                        )
                nc.scalar.activation(
                    out=qt_sb[:, c, :], in_=ps[:], func=Act.Identity,
                    scale=SQK / (SW * SX),
                    bias=bqr_sb[:, h * ET + c:h * ET + c + 1],
                )

            if h == 0:
                for j in range(SJT):
                    nc.sync.dma_start(out=xn_sb[:, j, :], in_=A["xn"][j * P:(j + 1) * P, :])

            # ---- attention: scores+softmax per si-tile, transposes pipelined
            # two tiles behind so the last softmax hides under the next scores
            m_sb = m_pool.tile([P, ET, SI], fp8, tag="m")
            attn_tiles = [None] * SIT
            at_tiles = [None] * 4

            def scores_softmax(t):
                a_t = attn_pool.tile([P, S], bf16, tag="attn")
                attn_tiles[t] = a_t
                r = red_pool.tile([P, 2], f32, tag="rsum")
                rec = red_pool.tile([P, 1], f32, tag="rec")
                for hs in range(2):
                    ps = psA.tile([P, 1024], f32, tag="psA")
                    for nb in range(2):
                        for kp in range(ET // 2):
                            nc.tensor.matmul(
                                ps[:, nb * 512:(nb + 1) * 512],
                                lhsT=qt_sb[:, 2 * kp:2 * kp + 2, t * P:(t + 1) * P],
                                rhs=kt_sb[:, 2 * kp:2 * kp + 2,
                                          hs * 1024 + nb * 512:hs * 1024 + (nb + 1) * 512],
                                start=(kp == 0), stop=(kp == ET // 2 - 1),
                                perf_mode=DR,
                            )
                    nc.scalar.activation(
                        out=a_t[:, hs * 1024:(hs + 1) * 1024], in_=ps[:],
                        func=Act.Exp, scale=SCALE / (SQK * SQK),
                        accum_out=r[:, hs:hs + 1],
                    )
                nc.vector.tensor_add(rec[:], r[:, 0:1], r[:, 1:2])
                nc.vector.reciprocal(rec[:], rec[:])
                nc.vector.tensor_scalar_mul(a_t[:], a_t[:], rec[:])

            def transposes(t):
                q, t2 = t // 2, t % 2
                if t2 == 0:
                    at_tiles[q] = atT_pool.tile(
                        [P, SJT, 256], fp8, tag="atT", name=f"at_q{q}"
                    )
                a_t = attn_tiles[t]
                for j8 in range(2):
                    pb = psB.tile([P, 1024], bf16, tag="psB")
                    for jj in range(8):
                        j = j8 * 8 + jj
                        nc.tensor.transpose(
                            pb[:, jj * P:(jj + 1) * P],
                            a_t[:, j * P:(j + 1) * P],
                            ident[:],
                        )
                    nc.vector.tensor_scalar_mul(
                        at_tiles[q][:, j8 * 8:(j8 + 1) * 8, t2 * P:(t2 + 1) * P],
                        pb[:].rearrange("p (j c) -> p j c", c=P),
                        SA,
                    )
                attn_tiles[t] = None

            def m_chains(q):
                at_sb = at_tiles[q]
                for dc in range(ET):
                    ps = psA.tile([P, 1024], f32, tag="psA")
                    for jp in range(SJT // 2):
                        nc.tensor.matmul(
                            ps[:, 0:256],
                            lhsT=xn_sb[:, 2 * jp:2 * jp + 2, dc * P:(dc + 1) * P],
                            rhs=at_sb[:, 2 * jp:2 * jp + 2, :],
                            start=(jp == 0), stop=(jp == SJT // 2 - 1),
                            perf_mode=DR,
                        )
                    nc.vector.tensor_scalar_mul(
                        m_sb[:, dc, q * 256:(q + 1) * 256], ps[:, 0:256],
                        SM / (SX * SA),
                    )

            scores_softmax(0)
            scores_softmax(1)
            for t in range(2, SIT):
                scores_softmax(t)
                transposes(t - 2)
                if t % 2 == 1:
                    m_chains((t - 2) // 2)
            transposes(SIT - 2)
            transposes(SIT - 1)
            m_chains(3)

            if h == H - 1:
                attn_ctx.close()

            # ---- head^T = Wv^T @ M : [e, si]
            ht_sb = ht_pool.tile([P, ET, SI], fp8, tag="ht")
            for eb in range(ET):
                wv_eb = wqkv_pool.tile([P, ET, P], fp8, tag="wqkv")
                nc.sync.dma_start(out=wv_eb[:], in_=A["wvb"][h, eb])
                ps = psA.tile([P, 1024], f32, tag="psA")
                for nb in range(2):
                    for kp in range(ET // 2):
                        nc.tensor.matmul(
                            ps[:, nb * 512:(nb + 1) * 512],
                            lhsT=wv_eb[:, 2 * kp:2 * kp + 2, :],
                            rhs=m_sb[:, 2 * kp:2 * kp + 2, nb * 512:(nb + 1) * 512],
                            start=(kp == 0), stop=(kp == ET // 2 - 1),
                            perf_mode=DR,
                        )
                nc.scalar.mul(ht_sb[:, eb, :], ps[:], SH / (SW * SM))

            w1_h = w1_pool.tile([P, ET, D], fp8, tag="w1", name=f"w1_{h}")
            nc.sync.dma_start(out=w1_h[:], in_=A["w1"][h])
            last_ht, last_w1 = ht_sb, w1_h

            if h == H - 1:
                mid_ctx.close()

            # ---- proj += head_h @ W1_h (head 3's chains interleave with LN)
            def w1_chain(t, ht_sb=ht_sb, w1_h=w1_h, h=h):
                ps = psA.tile([P, 1024], f32, tag="psA")
                for nb in range(2):
                    for ep in range(ET // 2):
                        nc.tensor.matmul(
                            ps[:, nb * 512:(nb + 1) * 512],
                            lhsT=ht_sb[:, 2 * ep:2 * ep + 2, t * P:(t + 1) * P],
                            rhs=w1_h[:, 2 * ep:2 * ep + 2, nb * 512:(nb + 1) * 512],
                            start=(ep == 0), stop=(ep == ET // 2 - 1),
                            perf_mode=DR,
                        )
                if h == 0:
                    nc.scalar.mul(proj_sb[:, t, :], ps[:], 1.0 / (SH * SW1))
                else:
                    nc.vector.scalar_tensor_tensor(
                        out=proj_sb[:, t, :], in0=ps[:], scalar=1.0 / (SH * SW1),
                        in1=proj_sb[:, t, :], op0=Alu.mult, op1=Alu.add,
                    )

            if h < H - 1:
                for t in range(SIT):
                    w1_chain(t)
            else:
                last_w1_chain = w1_chain

        # ================= LN1 -> FFN2 -> LN2, fully per-si-tile =================
        with ExitStack() as lctx:
            lnp = lctx.enter_context(tc.tile_pool(name="lnp", bufs=1))
            xr_pool = lctx.enter_context(tc.tile_pool(name="xr", bufs=3))
            u_pool = lctx.enter_context(tc.tile_pool(name="up", bufs=3))
            sq_pool = lctx.enter_context(tc.tile_pool(name="sq", bufs=2))
            ybf_pool = lctx.enter_context(tc.tile_pool(name="ybf", bufs=2))
            yt_pool = lctx.enter_context(tc.tile_pool(name="yt", bufs=3))
            w2_pool = lctx.enter_context(tc.tile_pool(name="w2", bufs=8))
            st_pool = lctx.enter_context(tc.tile_pool(name="st", bufs=8))
            ot_pool = lctx.enter_context(tc.tile_pool(name="ot", bufs=3))

            if not trivial_gbe:
                gbe_sb = lnp.tile([P, 4, D], f32, tag="gbe")
                gbe_bc = bass.AP(
                    tensor=A["gbe"].tensor, offset=A["gbe"].offset,
                    ap=[[0, P], A["gbe"].ap[0], A["gbe"].ap[1]],
                )
                nc.gpsimd.dma_start(out=gbe_sb[:], in_=gbe_bc)
            y_sb = lnp.tile([P, SIT, D], f32, tag="y")

            xr_tiles = []
            for t in range(SIT):
                xr = xr_pool.tile([P, D], f32, tag="xr", name=f"xr{t}")
                nc.sync.dma_start(out=xr[:], in_=A["xres"][t * P:(t + 1) * P, :])
                xr_tiles.append(xr)

            w2_tiles = []
            for kc in range(ET):
                w2_kc = w2_pool.tile([P, D], bf16, tag="w2")
                nc.sync.dma_start(out=w2_kc[:], in_=A["w2"][kc * P:(kc + 1) * P, :])
                w2_tiles.append(w2_kc)

            def ln_stats(src, rsum):
                """-> (mu, rstd) [P,1] tiles from src [P,D] + its row-sum."""
                sq = sq_pool.tile([P, D], f32, tag="sq")
                sumsq = st_pool.tile([P, 1], f32, tag="sumsq")
                nc.scalar.activation(out=sq[:], in_=src, func=Act.Square,
                                     accum_out=sumsq[:])
                mu = st_pool.tile([P, 1], f32, tag="mu")
                nc.scalar.mul(mu[:], rsum, 1.0 / D)
                # (rsum*mu - sumsq) = -D*var;  std = sqrt(-1/D * that + eps)
                nv = st_pool.tile([P, 1], f32, tag="nv")
                nc.vector.scalar_tensor_tensor(
                    out=nv[:], in0=rsum, scalar=mu[:], in1=sumsq[:],
                    op0=Alu.mult, op1=Alu.subtract,
                )
                rstd = st_pool.tile([P, 1], f32, tag="rstd")
                nc.scalar.activation(out=rstd[:], in_=nv[:], func=Act.Sqrt,
                                     scale=-1.0 / D, bias=eps_sb[:])
                nc.vector.reciprocal(rstd[:], rstd[:])
                return mu, rstd

            for t in range(SIT):
                # head 3's W1 chain for this tile runs just-in-time so the LN
                # chains of earlier tiles overlap it on vector/scalar
                last_w1_chain(t)
                # u1 = x + proj, with row-sum accumulated in the same pass
                u1 = u_pool.tile([P, D], f32, tag="u")
                rs1 = st_pool.tile([P, 1], f32, tag="rs")
                nc.vector.scalar_tensor_tensor(
                    out=u1[:], in0=xr_tiles[t][:], scalar=1.0,
                    in1=proj_sb[:, t, :], op0=Alu.mult, op1=Alu.add,
                    accum_out=rs1[:],
                )
                mu1, rstd1 = ln_stats(u1[:], rs1[:])
                yt_t = y_sb[:, t, :]
                nc.vector.tensor_scalar(
                    yt_t, u1[:], scalar1=mu1[:], scalar2=rstd1[:],
                    op0=Alu.subtract, op1=Alu.mult,
                )
                if not trivial_gbe:
                    nc.gpsimd.tensor_mul(yt_t, yt_t, gbe_sb[:, 0, :])
                    nc.gpsimd.tensor_add(yt_t, yt_t, gbe_sb[:, 1, :])
                yb = ybf_pool.tile([P, D], bf16, tag="ybf")
                nc.scalar.copy(yb[:], yt_t)
                # transpose this tile's 8 f-blocks -> yT columns for its z-chain
                yt_tile = yt_pool.tile([P, ET, P], bf16, tag="yt")
                pb = psB.tile([P, 1024], bf16, tag="psB")
                for fb in range(ET):
                    nc.tensor.transpose(
                        pb[:, fb * P:(fb + 1) * P], yb[:, fb * P:(fb + 1) * P], ident[:]
                    )
                nc.vector.tensor_copy(
                    yt_tile[:], pb[:].rearrange("p (f c) -> p f c", c=P)
                )
                # z-chain: u2 = y + yhat @ W2' + bu
                ps = psA.tile([P, 1024], f32, tag="psA")
                for nb in range(2):
                    for kc in range(ET):
                        nc.tensor.matmul(
                            ps[:, nb * 512:(nb + 1) * 512],
                            lhsT=yt_tile[:, kc, :],
                            rhs=w2_tiles[kc][:, nb * 512:(nb + 1) * 512],
                            start=(kc == 0), stop=False,
                        )
                    nc.tensor.matmul(
                        ps[:, nb * 512:(nb + 1) * 512],
                        lhsT=ones_sb[:, :],
                        rhs=buv_sb[:, nb * 512:(nb + 1) * 512],
                        start=False, stop=True,
                    )
                u2 = u_pool.tile([P, 1024], f32, tag="u")
                rs2 = st_pool.tile([P, 1], f32, tag="rs")
                nc.vector.scalar_tensor_tensor(
                    out=u2[:], in0=y_sb[:, t, :], scalar=1.0,
                    in1=ps[:], op0=Alu.mult, op1=Alu.add,
                    accum_out=rs2[:],
                )
                mu2, rstd2 = ln_stats(u2[:], rs2[:])
                ot = ot_pool.tile([P, D], f32, tag="ot")
                nc.vector.tensor_scalar(
                    ot[:], u2[:], scalar1=mu2[:], scalar2=rstd2[:],
                    op0=Alu.subtract, op1=Alu.mult,
                )
                if not trivial_gbe:
                    nc.gpsimd.tensor_mul(ot[:], ot[:], gbe_sb[:, 2, :])
                    nc.gpsimd.tensor_add(ot[:], ot[:], gbe_sb[:, 3, :])
                nc.sync.dma_start(out=A["out"][t * P:(t + 1) * P, :], in_=ot[:])

        tail_ctx.close()


def _build(trivial_gbe):
    import concourse.bass as bass
    import concourse.mybir as mybir
    import concourse.tile as tile
    from concourse import bacc

    f32 = mybir.dt.float32
    bf16 = mybir.dt.bfloat16
    fp8 = mybir.dt.float8e4

    nc = bacc.Bacc("TRN2", target_bir_lowering=False, debug=False, num_devices=8)
    A = {}

    def din(name, shape, dt):
        A[name] = nc.dram_tensor(name, shape, dt, kind="ExternalInput").ap()

    din("xt", [D, S], fp8)
    din("xn", [S, D], fp8)
    din("xres", [SI, D], f32)
    din("wqb", [H, ET, P, ET, P], fp8)
    din("wkb", [H, ET, P, ET, P], fp8)
    din("wvb", [H, ET, P, ET, P], fp8)
    din("w1", [H, P, ET, D], fp8)
    din("w2", [D, D], bf16)
    din("bqr", [P, H * ET], f32)
    din("buv", [1, D], bf16)
    if not trivial_gbe:
        din("gbe", [4, D], f32)
    A["out"] = nc.dram_tensor("out", [SI, D], f32, kind="ExternalOutput").ap()

    with tile.TileContext(nc) as tc:
        _emit(nc, tc, A, trivial_gbe)
    nc.compile()
    return nc


def _get_nc(trivial_gbe=True):
    key = ("nc", trivial_gbe)
    if key not in _CACHE:
        _CACHE[key] = _build(trivial_gbe)
    return _CACHE[key]


def _prep_inputs(inputs):
    x = np.ascontiguousarray(inputs["embedding_matrix"], dtype=np.float32)
    Wq = np.asarray(inputs["Wq"], np.float32)
    bq = np.asarray(inputs["bq"], np.float32)
    Wv = np.asarray(inputs["Wv"], np.float32)
    bv = np.asarray(inputs["bv"], np.float32)
    Wk = np.asarray(inputs["Wk"], np.float32)
    W1 = np.asarray(inputs["W1"], np.float32)
    b1 = np.asarray(inputs["b1"], np.float32)
    W2 = np.asarray(inputs["W2"], np.float32)
    b2 = np.asarray(inputs["b2"], np.float32)
    g1 = np.asarray(inputs["g1"], np.float32)
    be1 = np.asarray(inputs["be1"], np.float32)
    g2 = np.asarray(inputs["g2"], np.float32)
    be2 = np.asarray(inputs["be2"], np.float32)

    trivial = (
        np.array_equal(g1, np.ones(D, np.float32))
        and np.array_equal(g2, np.ones(D, np.float32))
        and np.array_equal(be1, np.zeros(D, np.float32))
        and np.array_equal(be2, np.zeros(D, np.float32))
    )

    def pack_w(W):  # [H, D, D] -> [H, ET, P(row-in-block), ET(kc), P] lhsT blocks
        return np.ascontiguousarray(
            (W * SW).reshape(H, ET, P, ET, P).transpose(0, 3, 2, 1, 4).astype(F8)
        )

    wqb = pack_w(Wq)
    wkb = pack_w(Wk)
    wvb = pack_w(Wv)
    # W1 [H*D, D] -> [H, P(p), ET(eb), D] fp8 lhsT-pair layout for the proj chain
    w1b = np.ascontiguousarray(
        (W1 * SW1).reshape(H, ET, P, D).transpose(0, 2, 1, 3).astype(F8)
    )
    w2b = np.ascontiguousarray(W2.astype(BF))
    # bq rearranged so bias for (h, e-block c) is column h*ET+c: [P, H*ET], x SQK
    bqr = np.ascontiguousarray(
        (bq * SQK).reshape(H, ET, P).transpose(2, 0, 1).reshape(P, H * ET)
    )
    cvec = (b1 + sum(bv[h] @ W1[h * D:(h + 1) * D] for h in range(H)))
    buv = np.ascontiguousarray(b2.reshape(1, D).astype(BF))

    shared = {
        "wqb": wqb, "wkb": wkb, "wvb": wvb, "w1": w1b, "w2": w2b,
        "bqr": bqr, "buv": buv,
    }
    if not trivial:
        shared["gbe"] = np.ascontiguousarray(np.stack([g1, be1, g2, be2]))
    in_maps = []
    for core in range(8):
        b, half = core // 2, core % 2
        own = x[b, half * SI:(half + 1) * SI]
        other = x[b, (1 - half) * SI:(2 - half) * SI]
        xperm = np.concatenate([own, other], axis=0)
        m = dict(shared)
        m["xn"] = np.ascontiguousarray((xperm * SX).astype(F8))
        m["xt"] = np.ascontiguousarray((xperm.T * SX).astype(F8))
        m["xres"] = np.ascontiguousarray(own + cvec[None, :])
        in_maps.append(m)
    return trivial, in_maps


def kernel(**inputs):
    from concourse.bass_utils import run_bass_kernel_spmd

    trivial, in_maps = _prep_inputs(inputs)
    nc = _get_nc(trivial)
    res = run_bass_kernel_spmd(nc, in_maps, core_ids=list(range(8)))
    out = np.empty((4, S, D), np.float32)
    for core in range(8):
        b, half = core // 2, core % 2
        out[b, half * SI:(half + 1) * SI] = res.results[core]["out"]
    return out


# revision 17
# speedup vs baseline: 1.0354x; 1.0354x over previous
"""Trainium2 Bass kernel for nn_EncoderOnlyBlock (4-head full-dim encoder block).

Sharding: fully data-parallel, no collectives. 8 cores = (batch b, seq-half).
Each core computes its 1024 query tokens end-to-end for all 4 heads; K work
for the full 2048-token batch row is recomputed on both cores of a batch
(the only duplicated work).

All heavy matmuls run in fp8-e4m3 DoubleRow mode (2 k-blocks per instruction,
2x bf16 throughput), with power-of-2 scale factors folded into the operands
so PSUM results are rescaled for free on the copy-out:
  x -> fp8 x*16;  Wq/Wk/Wv -> fp8 *4096;  W1 -> fp8 *512
  Q^T_h = Wq_h^T x^T (+16*bq via activation bias), stored fp8 = 16*Q
  K^T_h = Wk_h^T x^T stored fp8 = 16*K   (bk dropped: softmax invariant)
  S psum = 256*S;  A = exp(S/sqrtD) via activation scale 1/8192, bf16,
  normalized by 1/rowsum; A^T stored fp8 = 128*A (scaled in transpose copy)
  M_h = x^T A^T (psum 2048*M) stored fp8 = 32*M   (A@V == Wv^T@M reassoc,
  bv_h folded into cvec since rows of A sum to 1)
  hd^T_h = Wv_h^T M (psum 2^17*hd) stored fp8 = 16*hd
  proj = sum_h hd_h @ W1_h (psum 8192*proj) accumulated bf16 unscaled
  cvec = b1 + sum_h bv_h @ W1_h is folded into xres on the host.
  u1 = xres + proj;  yhat = LN1(u1);  u2 = y + yhat@W2' + bu  (bf16 chain)
  out = LN2(u2)
LN means/vars via sum & sum-of-squares accumulators (E[x^2]-mu^2); g1/be1 and
g2/be2 application is skipped when they are exactly ones/zeros (checked on
host; g1/be1 additionally fold into W2'/bu which is exact in that case).
"""

import numpy as np
import ml_dtypes

BF = ml_dtypes.bfloat16
F8 = ml_dtypes.float8_e4m3
P = 128
D = 1024
S = 2048
SI = 1024
H = 4
ET = D // P       # 8 e/d/f 128-blocks
SJT = S // P      # 16 sj 128-blocks
SIT = SI // P     # 8 si 128-blocks
SCALE = 1.0 / 32.0  # 1/sqrt(D)
EPS = 1e-5

SX = 16.0         # x fp8 scale
SW = 4096.0       # Wq/Wk/Wv fp8 scale
SW1 = 512.0       # W1 fp8 scale
SQK = 16.0        # Q/K fp8 storage scale
SA = 128.0        # A^T fp8 storage scale
SM = 32.0         # M fp8 storage scale
SH = 16.0         # head^T fp8 storage scale
SX2 = 16.0        # y fp8 scale for the z-chain
SW2 = 4096.0      # W2 fp8 scale

_CACHE = {}


def _emit(nc, tc, A, trivial_gbe):
    """Emit the per-core program. A: dict name -> dram AP."""
    from contextlib import ExitStack

    import concourse.bass as bass
    import concourse.mybir as mybir
    from concourse.masks import make_identity

    f32 = mybir.dt.float32
    bf16 = mybir.dt.bfloat16
    fp8 = mybir.dt.float8e4
    Act = mybir.ActivationFunctionType
    Alu = mybir.AluOpType
    DR = mybir.MatmulPerfMode.DoubleRow

    with ExitStack() as ctx:
        consts = ctx.enter_context(tc.tile_pool(name="consts", bufs=1))
        psA = ctx.enter_context(tc.tile_pool(name="psA", bufs=3, space="PSUM"))
        psB = ctx.enter_context(tc.tile_pool(name="psB", bufs=2, space="PSUM"))

        ident = consts.tile([P, P], bf16, tag="ident")
        make_identity(nc, ident[:])
        bqr_sb = consts.tile([P, H * ET], f32, tag="bqr")
        nc.sync.dma_start(out=bqr_sb[:], in_=A["bqr"][:])
        buv_sb = consts.tile([1, D], fp8, tag="buv")
        nc.sync.dma_start(out=buv_sb[:], in_=A["buv"][:])
        # 128 * (512*b2) = 65536*b2 matches the z-chain psum scale SX2*SW2
        ones_sb = consts.tile([1, P], fp8, tag="ones")
        nc.vector.memset(ones_sb[:], 128.0)
        eps_sb = consts.tile([P, 1], f32, tag="eps")
        nc.vector.memset(eps_sb[:], EPS)

        # attention-side pools close after the last m-chain; mid pools after the
        # last WvM; tail pools live through the interleaved W1(h3)+LN loop.
        tail_ctx = ExitStack()
        w1_pool = tail_ctx.enter_context(tc.tile_pool(name="w1", bufs=2))
        ht_pool = tail_ctx.enter_context(tc.tile_pool(name="ht", bufs=1))
        proj_pool = tail_ctx.enter_context(tc.tile_pool(name="pj", bufs=1))
        mid_ctx = ExitStack()
        wqkv_pool = mid_ctx.enter_context(tc.tile_pool(name="wqkv", bufs=3))
        m_pool = mid_ctx.enter_context(tc.tile_pool(name="m", bufs=1))
        attn_ctx = ExitStack()
        xpool = attn_ctx.enter_context(tc.tile_pool(name="xp", bufs=1))
        qt_pool = attn_ctx.enter_context(tc.tile_pool(name="qt", bufs=1))
        kt_pool = attn_ctx.enter_context(tc.tile_pool(name="kt", bufs=1))
        attn_pool = attn_ctx.enter_context(tc.tile_pool(name="at", bufs=3))
        atT_pool = attn_ctx.enter_context(tc.tile_pool(name="atT", bufs=1))
        red_pool = attn_ctx.enter_context(tc.tile_pool(name="red", bufs=8))

        # head-0 c=0 K weights first (the first chain's LDWEIGHTS needs them),
        # then x^T low halves (hs=0 chains), then high halves; x natural waits
        # until head-0's weights are queued (not needed until the M phase)
        wk00 = wqkv_pool.tile([P, ET, P], fp8, tag="wqkv")
        nc.sync.dma_start(out=wk00[:], in_=A["wkb"][0, 0])
        xt_sb = xpool.tile([P, ET, S], fp8, tag="xt")
        for hs in range(2):
            for c in range(ET):
                nc.sync.dma_start(
                    out=xt_sb[:, c, hs * 1024:(hs + 1) * 1024],
                    in_=A["xt"][c * P:(c + 1) * P, hs * 1024:(hs + 1) * 1024],
                )
        xn_sb = xpool.tile([P, SJT, D], fp8, tag="xn")

        proj_sb = proj_pool.tile([P, SIT, D], bf16, tag="proj")

        for h in range(H):
            # ---- K^T = Wk^T @ x^T : [e, sj], fp8 DoubleRow over kc pairs
            kt_sb = kt_pool.tile([P, ET, S], fp8, tag="kt")
            for c in range(ET):
                if h == 0 and c == 0:
                    wk_c = wk00
                else:
                    wk_c = wqkv_pool.tile([P, ET, P], fp8, tag="wqkv")
                    nc.sync.dma_start(out=wk_c[:], in_=A["wkb"][h, c])
                for hs in range(2):
                    ps = psA.tile([P, 1024], f32, tag="psA")
                    for nb in range(2):
                        for kp in range(ET // 2):
                            nc.tensor.matmul(
                                ps[:, nb * 512:(nb + 1) * 512],
                                lhsT=wk_c[:, 2 * kp:2 * kp + 2, :],
                                rhs=xt_sb[:, 2 * kp:2 * kp + 2,
                                          hs * 1024 + nb * 512:hs * 1024 + (nb + 1) * 512],
                                start=(kp == 0), stop=(kp == ET // 2 - 1),
                                perf_mode=DR,
                            )
                    nc.scalar.mul(kt_sb[:, c, hs * 1024:(hs + 1) * 1024], ps[:],
                                  SQK / (SW * SX))

            # ---- Q^T = Wq^T @ x^T[:, :1024] + bq : [e, si]
            qt_sb = qt_pool.tile([P, ET, SI], fp8, tag="qt")
            for c in range(ET):
                wq_c = wqkv_pool.tile([P, ET, P], fp8, tag="wqkv")
                nc.sync.dma_start(out=wq_c[:], in_=A["wqb"][h, c])
                ps = psA.tile([P, 1024], f32, tag="psA")
                for nb in range(2):
                    for kp in range(ET // 2):
                        nc.tensor.matmul(
                            ps[:, nb * 512:(nb + 1) * 512],
                            lhsT=wq_c[:, 2 * kp:2 * kp + 2, :],
                            rhs=xt_sb[:, 2 * kp:2 * kp + 2, nb * 512:(nb + 1) * 512],
                            start=(kp == 0), stop=(kp == ET // 2 - 1),
                            perf_mode=DR,
                        )
                nc.scalar.activation(
                    out=qt_sb[:, c, :], in_=ps[:], func=Act.Identity,
                    scale=SQK / (SW * SX),
                    bias=bqr_sb[:, h * ET + c:h * ET + c + 1],
                )

            if h == 0:
                for j in range(SJT):
                    nc.sync.dma_start(out=xn_sb[:, j, :], in_=A["xn"][j * P:(j + 1) * P, :])

            # ---- attention: scores+softmax per si-tile, transposes pipelined
            # two tiles behind so the last softmax hides under the next scores
            m_sb = m_pool.tile([P, ET, SI], fp8, tag="m")
            attn_tiles = [None] * SIT
            at_tiles = [None] * 4

            def scores_softmax(t):
                a_t = attn_pool.tile([P, S], bf16, tag="attn")
                attn_tiles[t] = a_t
                r = red_pool.tile([P, 2], f32, tag="rsum")
                rec = red_pool.tile([P, 1], f32, tag="rec")
                for hs in range(2):
                    ps = psA.tile([P, 1024], f32, tag="psA")
                    for nb in range(2):
                        for kp in range(ET // 2):
                            nc.tensor.matmul(
                                ps[:, nb * 512:(nb + 1) * 512],
                                lhsT=qt_sb[:, 2 * kp:2 * kp + 2, t * P:(t + 1) * P],
                                rhs=kt_sb[:, 2 * kp:2 * kp + 2,
                                          hs * 1024 + nb * 512:hs * 1024 + (nb + 1) * 512],
                                start=(kp == 0), stop=(kp == ET // 2 - 1),
                                perf_mode=DR,
                            )
                    nc.scalar.activation(
                        out=a_t[:, hs * 1024:(hs + 1) * 1024], in_=ps[:],
                        func=Act.Exp, scale=SCALE / (SQK * SQK),
                        accum_out=r[:, hs:hs + 1],
                    )
                nc.vector.tensor_add(rec[:], r[:, 0:1], r[:, 1:2])
                nc.vector.reciprocal(rec[:], rec[:])
                nc.vector.tensor_scalar_mul(a_t[:], a_t[:], rec[:])

            def transposes(t):
                q, t2 = t // 2, t % 2
                if t2 == 0:
                    at_tiles[q] = atT_pool.tile(
                        [P, SJT, 256], fp8, tag="atT", name=f"at_q{q}"
                    )
                a_t = attn_tiles[t]
                for j8 in range(2):
                    pb = psB.tile([P, 1024], bf16, tag="psB")
                    for jj in range(8):
                        j = j8 * 8 + jj
                        nc.tensor.transpose(
                            pb[:, jj * P:(jj + 1) * P],
                            a_t[:, j * P:(j + 1) * P],
                            ident[:],
                        )
                    nc.vector.tensor_scalar_mul(
                        at_tiles[q][:, j8 * 8:(j8 + 1) * 8, t2 * P:(t2 + 1) * P],
                        pb[:].rearrange("p (j c) -> p j c", c=P),
                        SA,
                    )
                attn_tiles[t] = None

            def m_chains(q):
                at_sb = at_tiles[q]
                for dc in range(ET):
                    ps = psA.tile([P, 1024], f32, tag="psA")
                    for jp in range(SJT // 2):
                        nc.tensor.matmul(
                            ps[:, 0:256],
                            lhsT=xn_sb[:, 2 * jp:2 * jp + 2, dc * P:(dc + 1) * P],
                            rhs=at_sb[:, 2 * jp:2 * jp + 2, :],
                            start=(jp == 0), stop=(jp == SJT // 2 - 1),
                            perf_mode=DR,
                        )
                    nc.vector.tensor_scalar_mul(
                        m_sb[:, dc, q * 256:(q + 1) * 256], ps[:, 0:256],
                        SM / (SX * SA),
                    )

            scores_softmax(0)
            scores_softmax(1)
            for t in range(2, SIT):
                scores_softmax(t)
                transposes(t - 2)
                if t % 2 == 1:
                    m_chains((t - 2) // 2)
            transposes(SIT - 2)
            transposes(SIT - 1)
            m_chains(3)

            if h == H - 1:
                attn_ctx.close()

            # ---- head^T = Wv^T @ M : [e, si]
            ht_sb = ht_pool.tile([P, ET, SI], fp8, tag="ht")
            for eb in range(ET):
                wv_eb = wqkv_pool.tile([P, ET, P], fp8, tag="wqkv")
                nc.sync.dma_start(out=wv_eb[:], in_=A["wvb"][h, eb])
                ps = psA.tile([P, 1024], f32, tag="psA")
                for nb in range(2):
                    for kp in range(ET // 2):
                        nc.tensor.matmul(
                            ps[:, nb * 512:(nb + 1) * 512],
                            lhsT=wv_eb[:, 2 * kp:2 * kp + 2, :],
                            rhs=m_sb[:, 2 * kp:2 * kp + 2, nb * 512:(nb + 1) * 512],
                            start=(kp == 0), stop=(kp == ET // 2 - 1),
                            perf_mode=DR,
                        )
                nc.scalar.mul(ht_sb[:, eb, :], ps[:], SH / (SW * SM))

            w1_h = w1_pool.tile([P, ET, D], fp8, tag="w1", name=f"w1_{h}")
            nc.sync.dma_start(out=w1_h[:], in_=A["w1"][h])
            last_ht, last_w1 = ht_sb, w1_h

            if h == H - 1:
                mid_ctx.close()

            # ---- proj += head_h @ W1_h (head 3's chains interleave with LN)
            def w1_chain(t, ht_sb=ht_sb, w1_h=w1_h, h=h):
                ps = psA.tile([P, 1024], f32, tag="psA")
                for nb in range(2):
                    for ep in range(ET // 2):
                        nc.tensor.matmul(
                            ps[:, nb * 512:(nb + 1) * 512],
                            lhsT=ht_sb[:, 2 * ep:2 * ep + 2, t * P:(t + 1) * P],
                            rhs=w1_h[:, 2 * ep:2 * ep + 2, nb * 512:(nb + 1) * 512],
                            start=(ep == 0), stop=(ep == ET // 2 - 1),
                            perf_mode=DR,
                        )
                if h == 0:
                    nc.scalar.mul(proj_sb[:, t, :], ps[:], 1.0 / (SH * SW1))
                else:
                    nc.vector.scalar_tensor_tensor(
                        out=proj_sb[:, t, :], in0=ps[:], scalar=1.0 / (SH * SW1),
                        in1=proj_sb[:, t, :], op0=Alu.mult, op1=Alu.add,
                    )

            if h < H - 1:
                for t in range(SIT):
                    w1_chain(t)
            else:
                last_w1_chain = w1_chain

        # ================= LN1 -> FFN2 -> LN2, fully per-si-tile =================
        with ExitStack() as lctx:
            lnp = lctx.enter_context(tc.tile_pool(name="lnp", bufs=1))
            xr_pool = lctx.enter_context(tc.tile_pool(name="xr", bufs=3))
            u_pool = lctx.enter_context(tc.tile_pool(name="up", bufs=3))
            sq_pool = lctx.enter_context(tc.tile_pool(name="sq", bufs=2))
            ybf_pool = lctx.enter_context(tc.tile_pool(name="ybf", bufs=4))
            yt_pool = lctx.enter_context(tc.tile_pool(name="yt", bufs=3))
            w2_pool = lctx.enter_context(tc.tile_pool(name="w2", bufs=8))
            st_pool = lctx.enter_context(tc.tile_pool(name="st", bufs=8))
            ot_pool = lctx.enter_context(tc.tile_pool(name="ot", bufs=3))

            if not trivial_gbe:
                gbe_sb = lnp.tile([P, 4, D], f32, tag="gbe")
                gbe_bc = bass.AP(
                    tensor=A["gbe"].tensor, offset=A["gbe"].offset,
                    ap=[[0, P], A["gbe"].ap[0], A["gbe"].ap[1]],
                )
                nc.gpsimd.dma_start(out=gbe_sb[:], in_=gbe_bc)
            xr_tiles = []
            for t in range(SIT):
                xr = xr_pool.tile([P, D], f32, tag="xr", name=f"xr{t}")
                nc.sync.dma_start(out=xr[:], in_=A["xres"][t * P:(t + 1) * P, :])
                xr_tiles.append(xr)

            w2_sb = lnp.tile([P, ET, D], fp8, tag="w2")
            nc.sync.dma_start(out=w2_sb[:], in_=A["w2"][:])

            def ln_stats(src, rsum):
                """-> (mu, rstd) [P,1] tiles from src [P,D] + its row-sum."""
                sq = sq_pool.tile([P, D], f32, tag="sq")
                sumsq = st_pool.tile([P, 1], f32, tag="sumsq")
                nc.scalar.activation(out=sq[:], in_=src, func=Act.Square,
                                     accum_out=sumsq[:])
                mu = st_pool.tile([P, 1], f32, tag="mu")
                nc.scalar.mul(mu[:], rsum, 1.0 / D)
                # (rsum*mu - sumsq) = -D*var;  std = sqrt(-1/D * that + eps)
                nv = st_pool.tile([P, 1], f32, tag="nv")
                nc.vector.scalar_tensor_tensor(
                    out=nv[:], in0=rsum, scalar=mu[:], in1=sumsq[:],
                    op0=Alu.mult, op1=Alu.subtract,
                )
                rstd = st_pool.tile([P, 1], f32, tag="rstd")
                nc.scalar.activation(out=rstd[:], in_=nv[:], func=Act.Sqrt,
                                     scale=-1.0 / D, bias=eps_sb[:])
                nc.vector.reciprocal(rstd[:], rstd[:])
                return mu, rstd

            # head-3 W1 chains run 2 tiles ahead of the LN chains on the tensor
            # queue so LN1's serial vector/scalar latency hides under them
            last_w1_chain(0)
            last_w1_chain(1)
            y_tiles = [None] * SIT
            for t in range(SIT):
                if t + 2 < SIT:
                    last_w1_chain(t + 2)
                # u1 = x + proj, with row-sum accumulated in the same pass
                u1 = u_pool.tile([P, D], f32, tag="u")
                rs1 = st_pool.tile([P, 1], f32, tag="rs")
                nc.vector.scalar_tensor_tensor(
                    out=u1[:], in0=xr_tiles[t][:], scalar=1.0,
                    in1=proj_sb[:, t, :], op0=Alu.mult, op1=Alu.add,
                    accum_out=rs1[:],
                )
                mu1, rstd1 = ln_stats(u1[:], rs1[:])
                yb = ybf_pool.tile([P, D], bf16, tag="ybf")
                y_tiles[t] = yb
                nc.vector.tensor_scalar(
                    yb[:], u1[:], scalar1=mu1[:], scalar2=rstd1[:],
                    op0=Alu.subtract, op1=Alu.mult,
                )
                if not trivial_gbe:
                    nc.gpsimd.tensor_mul(yb[:], yb[:], gbe_sb[:, 0, :])
                    nc.gpsimd.tensor_add(yb[:], yb[:], gbe_sb[:, 1, :])
                # transpose this tile's 8 f-blocks -> yT columns for its z-chain
                yt_tile = yt_pool.tile([P, ET, P], fp8, tag="yt")
                pb = psB.tile([P, 1024], bf16, tag="psB")
                for fb in range(ET):
                    nc.tensor.transpose(
                        pb[:, fb * P:(fb + 1) * P], yb[:, fb * P:(fb + 1) * P], ident[:]
                    )
                nc.vector.tensor_scalar_mul(
                    yt_tile[:], pb[:].rearrange("p (f c) -> p f c", c=P), SX2,
                )
                # z-chain: u2 = y + y @ W2 + b2, fp8 DoubleRow (psum = SX2*SW2*z)
                ps = psA.tile([P, 1024], f32, tag="psA")
                for nb in range(2):
                    for kp in range(ET // 2):
                        nc.tensor.matmul(
                            ps[:, nb * 512:(nb + 1) * 512],
                            lhsT=yt_tile[:, 2 * kp:2 * kp + 2, :],
                            rhs=w2_sb[:, 2 * kp:2 * kp + 2, nb * 512:(nb + 1) * 512],
                            start=(kp == 0), stop=False,
                            perf_mode=DR,
                        )
                    nc.tensor.matmul(
                        ps[:, nb * 512:(nb + 1) * 512],
                        lhsT=ones_sb[:, :],
                        rhs=buv_sb[:, nb * 512:(nb + 1) * 512],
                        start=False, stop=True,
                    )
                u2 = u_pool.tile([P, 1024], f32, tag="u")
                rs2 = st_pool.tile([P, 1], f32, tag="rs")
                nc.vector.scalar_tensor_tensor(
                    out=u2[:], in0=ps[:], scalar=1.0 / (SX2 * SW2),
                    in1=y_tiles[t][:], op0=Alu.mult, op1=Alu.add,
                    accum_out=rs2[:],
                )
                mu2, rstd2 = ln_stats(u2[:], rs2[:])
                ot = ot_pool.tile([P, D], f32, tag="ot")
                nc.vector.tensor_scalar(
                    ot[:], u2[:], scalar1=mu2[:], scalar2=rstd2[:],
                    op0=Alu.subtract, op1=Alu.mult,
                )
                if not trivial_gbe:
                    nc.gpsimd.tensor_mul(ot[:], ot[:], gbe_sb[:, 2, :])
                    nc.gpsimd.tensor_add(ot[:], ot[:], gbe_sb[:, 3, :])
                nc.sync.dma_start(out=A["out"][t * P:(t + 1) * P, :], in_=ot[:])

        tail_ctx.close()


def _build(trivial_gbe):
    import concourse.bass as bass
    import concourse.mybir as mybir
    import concourse.tile as tile
    from concourse import bacc

    f32 = mybir.dt.float32
    bf16 = mybir.dt.bfloat16
    fp8 = mybir.dt.float8e4

    nc = bacc.Bacc("TRN2", target_bir_lowering=False, debug=False, num_devices=8)
    A = {}

    def din(name, shape, dt):
        A[name] = nc.dram_tensor(name, shape, dt, kind="ExternalInput").ap()

    din("xt", [D, S], fp8)
    din("xn", [S, D], fp8)
    din("xres", [SI, D], f32)
    din("wqb", [H, ET, P, ET, P], fp8)
    din("wkb", [H, ET, P, ET, P], fp8)
    din("wvb", [H, ET, P, ET, P], fp8)
    din("w1", [H, P, ET, D], fp8)
    din("w2", [P, ET, D], fp8)
    din("bqr", [P, H * ET], f32)
    din("buv", [1, D], fp8)
    if not trivial_gbe:
        din("gbe", [4, D], f32)
    A["out"] = nc.dram_tensor("out", [SI, D], f32, kind="ExternalOutput").ap()

    with tile.TileContext(nc) as tc:
        _emit(nc, tc, A, trivial_gbe)
    nc.compile()
    return nc


def _get_nc(trivial_gbe=True):
    key = ("nc", trivial_gbe)
    if key not in _CACHE:
        _CACHE[key] = _build(trivial_gbe)
    return _CACHE[key]


def _prep_inputs(inputs):
    x = np.ascontiguousarray(inputs["embedding_matrix"], dtype=np.float32)
    Wq = np.asarray(inputs["Wq"], np.float32)
    bq = np.asarray(inputs["bq"], np.float32)
    Wv = np.asarray(inputs["Wv"], np.float32)
    bv = np.asarray(inputs["bv"], np.float32)
    Wk = np.asarray(inputs["Wk"], np.float32)
    W1 = np.asarray(inputs["W1"], np.float32)
    b1 = np.asarray(inputs["b1"], np.float32)
    W2 = np.asarray(inputs["W2"], np.float32)
    b2 = np.asarray(inputs["b2"], np.float32)
    g1 = np.asarray(inputs["g1"], np.float32)
    be1 = np.asarray(inputs["be1"], np.float32)
    g2 = np.asarray(inputs["g2"], np.float32)
    be2 = np.asarray(inputs["be2"], np.float32)

    trivial = (
        np.array_equal(g1, np.ones(D, np.float32))
        and np.array_equal(g2, np.ones(D, np.float32))
        and np.array_equal(be1, np.zeros(D, np.float32))
        and np.array_equal(be2, np.zeros(D, np.float32))
    )

    def pack_w(W):  # [H, D, D] -> [H, ET, P(row-in-block), ET(kc), P] lhsT blocks
        return np.ascontiguousarray(
            (W * SW).reshape(H, ET, P, ET, P).transpose(0, 3, 2, 1, 4).astype(F8)
        )

    wqb = pack_w(Wq)
    wkb = pack_w(Wk)
    wvb = pack_w(Wv)
    # W1 [H*D, D] -> [H, P(p), ET(eb), D] fp8 lhsT-pair layout for the proj chain
    w1b = np.ascontiguousarray(
        (W1 * SW1).reshape(H, ET, P, D).transpose(0, 2, 1, 3).astype(F8)
    )
    # W2 [D, D] -> [P(p), ET(kc), D] fp8 lhsT-pair layout for the z-chain
    w2b = np.ascontiguousarray(
        (W2 * SW2).reshape(ET, P, D).transpose(1, 0, 2).astype(F8)
    )
    # bq rearranged so bias for (h, e-block c) is column h*ET+c: [P, H*ET], x SQK
    bqr = np.ascontiguousarray(
        (bq * SQK).reshape(H, ET, P).transpose(2, 0, 1).reshape(P, H * ET)
    )
    cvec = (b1 + sum(bv[h] @ W1[h * D:(h + 1) * D] for h in range(H)))
    # rank-1 ones(=128) x buv(=512*b2) adds SX2*SW2*b2 to the z-chain psum
    buv = np.ascontiguousarray((b2 * 512.0).reshape(1, D).astype(F8))

    shared = {
        "wqb": wqb, "wkb": wkb, "wvb": wvb, "w1": w1b, "w2": w2b,
        "bqr": bqr, "buv": buv,
    }
    if not trivial:
        shared["gbe"] = np.ascontiguousarray(np.stack([g1, be1, g2, be2]))
    in_maps = []
    for core in range(8):
        b, half = core // 2, core % 2
        own = x[b, half * SI:(half + 1) * SI]
        other = x[b, (1 - half) * SI:(2 - half) * SI]
        xperm = np.concatenate([own, other], axis=0)
        m = dict(shared)
        m["xn"] = np.ascontiguousarray((xperm * SX).astype(F8))
        m["xt"] = np.ascontiguousarray((xperm.T * SX).astype(F8))
        m["xres"] = np.ascontiguousarray(own + cvec[None, :])
        in_maps.append(m)
    return trivial, in_maps


def kernel(**inputs):
    from concourse.bass_utils import run_bass_kernel_spmd

    trivial, in_maps = _prep_inputs(inputs)
    nc = _get_nc(trivial)
    res = run_bass_kernel_spmd(nc, in_maps, core_ids=list(range(8)))
    out = np.empty((4, S, D), np.float32)
    for core in range(8):
        b, half = core // 2, core % 2
        out[b, half * SI:(half + 1) * SI] = res.results[core]["out"]
    return out


# revision 28
# speedup vs baseline: 1.1084x; 1.0704x over previous
"""Trainium2 Bass kernel for nn_EncoderOnlyBlock (4-head full-dim encoder block).

Sharding: fully data-parallel, no collectives. 8 cores = (batch b, seq-half).
Each core computes its 1024 query tokens end-to-end for all 4 heads; K work
for the full 2048-token batch row is recomputed on both cores of a batch
(the only duplicated work).

All heavy matmuls run in fp8-e4m3 DoubleRow mode (2 k-blocks per instruction,
2x bf16 throughput), with power-of-2 scale factors folded into the operands
so PSUM results are rescaled for free on the copy-out:
  x -> fp8 x*16;  Wq/Wk/Wv -> fp8 *4096;  W1 -> fp8 *512
  Q^T_h = Wq_h^T x^T (+16*bq via activation bias), stored fp8 = 16*Q
  K^T_h = Wk_h^T x^T stored fp8 = 16*K   (bk dropped: softmax invariant)
  S psum = 256*S;  A = exp(S/sqrtD) via activation scale 1/8192, bf16,
  normalized by 1/rowsum; A^T stored fp8 = 128*A (scaled in transpose copy)
  M_h = x^T A^T (psum 2048*M) stored fp8 = 32*M   (A@V == Wv^T@M reassoc,
  bv_h folded into cvec since rows of A sum to 1)
  hd^T_h = Wv_h^T M (psum 2^17*hd) stored fp8 = 16*hd
  proj = sum_h hd_h @ W1_h (psum 8192*proj) accumulated bf16 unscaled
  cvec = b1 + sum_h bv_h @ W1_h is folded into xres on the host.
  u1 = xres + proj;  yhat = LN1(u1);  u2 = y + yhat@W2' + bu  (bf16 chain)
  out = LN2(u2)
LN means/vars via sum & sum-of-squares accumulators (E[x^2]-mu^2); g1/be1 and
g2/be2 application is skipped when they are exactly ones/zeros (checked on
host; g1/be1 additionally fold into W2'/bu which is exact in that case).
"""

import numpy as np
import ml_dtypes

BF = ml_dtypes.bfloat16
F8 = ml_dtypes.float8_e4m3
P = 128
D = 1024
S = 2048
SI = 1024
H = 4
ET = D // P       # 8 e/d/f 128-blocks
SJT = S // P      # 16 sj 128-blocks
SIT = SI // P     # 8 si 128-blocks
SCALE = 1.0 / 32.0  # 1/sqrt(D)
EPS = 1e-5

SX = 16.0         # x fp8 scale
SW = 4096.0       # Wq/Wk/Wv fp8 scale
SW1 = 512.0       # W1 fp8 scale
SQK = 16.0        # Q/K fp8 storage scale
SA = 128.0        # A^T fp8 storage scale
SM = 32.0         # M fp8 storage scale
SH = 16.0         # head^T fp8 storage scale

_CACHE = {}


def _emit(nc, tc, A, trivial_gbe):
    """Emit the per-core program. A: dict name -> dram AP."""
    from contextlib import ExitStack

    import concourse.bass as bass
    import concourse.mybir as mybir
    from concourse.masks import make_identity

    f32 = mybir.dt.float32
    bf16 = mybir.dt.bfloat16
    fp8 = mybir.dt.float8e4
    Act = mybir.ActivationFunctionType
    Alu = mybir.AluOpType
    DR = mybir.MatmulPerfMode.DoubleRow

    with ExitStack() as ctx:
        consts = ctx.enter_context(tc.tile_pool(name="consts", bufs=1))
        psA = ctx.enter_context(tc.tile_pool(name="psA", bufs=3, space="PSUM"))
        psB = ctx.enter_context(tc.tile_pool(name="psB", bufs=2, space="PSUM"))

        ident = consts.tile([P, P], bf16, tag="ident")
        make_identity(nc, ident[:])
        bqr_sb = consts.tile([P, H * ET], f32, tag="bqr")
        nc.sync.dma_start(out=bqr_sb[:], in_=A["bqr"][:])
        buv_sb = consts.tile([1, D], bf16, tag="buv")
        nc.sync.dma_start(out=buv_sb[:], in_=A["buv"][:])
        ones_sb = consts.tile([1, P], bf16, tag="ones")
        nc.vector.memset(ones_sb[:], 1.0)
        eps_sb = consts.tile([P, 1], f32, tag="eps")
        nc.vector.memset(eps_sb[:], EPS)

        # attention-side pools close after the last m-chain; mid pools after the
        # last WvM; tail pools live through the interleaved W1(h3)+LN loop.
        tail_ctx = ExitStack()
        w1_pool = tail_ctx.enter_context(tc.tile_pool(name="w1", bufs=2))
        ht_pool = tail_ctx.enter_context(tc.tile_pool(name="ht", bufs=1))
        proj_pool = tail_ctx.enter_context(tc.tile_pool(name="pj", bufs=1))
        mid_ctx = ExitStack()
        wqkv_pool = mid_ctx.enter_context(tc.tile_pool(name="wqkv", bufs=3))
        m_pool = mid_ctx.enter_context(tc.tile_pool(name="m", bufs=1))
        attn_ctx = ExitStack()
        xpool = attn_ctx.enter_context(tc.tile_pool(name="xp", bufs=1))
        qt_pool = attn_ctx.enter_context(tc.tile_pool(name="qt", bufs=1))
        kt_pool = attn_ctx.enter_context(tc.tile_pool(name="kt", bufs=1))
        attn_pool = attn_ctx.enter_context(tc.tile_pool(name="at", bufs=3))
        atT_pool = attn_ctx.enter_context(tc.tile_pool(name="atT", bufs=1))
        red_pool = attn_ctx.enter_context(tc.tile_pool(name="red", bufs=8))

        # head-0 c=0 K weights first (the first chain's LDWEIGHTS needs them),
        # then x^T low halves (hs=0 chains), then high halves; x natural waits
        # until head-0's weights are queued (not needed until the M phase)
        wk_next = wqkv_pool.tile([P, ET, P], fp8, tag="wqkv", name="wk_pre0")
        nc.sync.dma_start(out=wk_next[:], in_=A["wkb"][0, 0])
        xt_sb = xpool.tile([P, ET, S], fp8, tag="xt")
        xt_src = A["xt"].rearrange("(c p) s -> p c s", p=P)
        for hs in range(2):
            nc.sync.dma_start(
                out=xt_sb[:, :, hs * 1024:(hs + 1) * 1024],
                in_=xt_src[:, :, hs * 1024:(hs + 1) * 1024],
            )
        xn_sb = xpool.tile([P, SJT, D], fp8, tag="xn")

        proj_sb = proj_pool.tile([P, SIT, D], bf16, tag="proj")

        for h in range(H):
            # ---- K^T = Wk^T @ x^T : [e, sj], fp8 DoubleRow over kc pairs
            kt_sb = kt_pool.tile([P, ET, S], fp8, tag="kt")
            for c in range(ET):
                if c == 0:
                    wk_c = wk_next
                else:
                    wk_c = wqkv_pool.tile([P, ET, P], fp8, tag="wqkv")
                    nc.sync.dma_start(out=wk_c[:], in_=A["wkb"][h, c])
                for hs in range(2):
                    ps = psA.tile([P, 1024], f32, tag="psA")
                    for nb in range(2):
                        for kp in range(ET // 2):
                            nc.tensor.matmul(
                                ps[:, nb * 512:(nb + 1) * 512],
                                lhsT=wk_c[:, 2 * kp:2 * kp + 2, :],
                                rhs=xt_sb[:, 2 * kp:2 * kp + 2,
                                          hs * 1024 + nb * 512:hs * 1024 + (nb + 1) * 512],
                                start=(kp == 0), stop=(kp == ET // 2 - 1),
                                perf_mode=DR,
                            )
                    nc.scalar.mul(kt_sb[:, c, hs * 1024:(hs + 1) * 1024], ps[:],
                                  SQK / (SW * SX))

            # ---- Q^T = Wq^T @ x^T[:, :1024] + bq : [e, si]
            qt_sb = qt_pool.tile([P, ET, SI], fp8, tag="qt")
            for c in range(ET):
                wq_c = wqkv_pool.tile([P, ET, P], fp8, tag="wqkv")
                nc.sync.dma_start(out=wq_c[:], in_=A["wqb"][h, c])
                ps = psA.tile([P, 1024], f32, tag="psA")
                for nb in range(2):
                    for kp in range(ET // 2):
                        nc.tensor.matmul(
                            ps[:, nb * 512:(nb + 1) * 512],
                            lhsT=wq_c[:, 2 * kp:2 * kp + 2, :],
                            rhs=xt_sb[:, 2 * kp:2 * kp + 2, nb * 512:(nb + 1) * 512],
                            start=(kp == 0), stop=(kp == ET // 2 - 1),
                            perf_mode=DR,
                        )
                nc.scalar.activation(
                    out=qt_sb[:, c, :], in_=ps[:], func=Act.Identity,
                    scale=SQK / (SW * SX),
                    bias=bqr_sb[:, h * ET + c:h * ET + c + 1],
                )

            if h == 0:
                nc.sync.dma_start(
                    out=xn_sb[:], in_=A["xn"].rearrange("(j p) d -> p j d", p=P)
                )

            # ---- attention: scores+softmax per si-tile, transposes pipelined
            # two tiles behind so the last softmax hides under the next scores
            m_sb = m_pool.tile([P, ET, SI], fp8, tag="m")
            attn_tiles = [None] * SIT
            at_tiles = [None] * 4

            def scores_softmax(t):
                a_t = attn_pool.tile([P, S], bf16, tag="attn")
                attn_tiles[t] = a_t
                r = red_pool.tile([P, 2], f32, tag="rsum")
                rec = red_pool.tile([P, 1], f32, tag="rec")
                for hs in range(2):
                    ps = psA.tile([P, 1024], f32, tag="psA")
                    for nb in range(2):
                        for kp in range(ET // 2):
                            nc.tensor.matmul(
                                ps[:, nb * 512:(nb + 1) * 512],
                                lhsT=qt_sb[:, 2 * kp:2 * kp + 2, t * P:(t + 1) * P],
                                rhs=kt_sb[:, 2 * kp:2 * kp + 2,
                                          hs * 1024 + nb * 512:hs * 1024 + (nb + 1) * 512],
                                start=(kp == 0), stop=(kp == ET // 2 - 1),
                                perf_mode=DR,
                            )
                    nc.scalar.activation(
                        out=a_t[:, hs * 1024:(hs + 1) * 1024], in_=ps[:],
                        func=Act.Exp, scale=SCALE / (SQK * SQK),
                        accum_out=r[:, hs:hs + 1],
                    )
                nc.vector.tensor_add(rec[:], r[:, 0:1], r[:, 1:2])
                nc.vector.reciprocal(rec[:], rec[:])
                nc.vector.tensor_scalar_mul(a_t[:], a_t[:], rec[:])

            def transposes(t):
                q, t2 = t // 2, t % 2
                if t2 == 0:
                    at_tiles[q] = atT_pool.tile(
                        [P, SJT, 256], fp8, tag="atT", name=f"at_q{q}"
                    )
                a_t = attn_tiles[t]
                for j8 in range(2):
                    pb = psB.tile([P, 1024], bf16, tag="psB")
                    for jj in range(8):
                        j = j8 * 8 + jj
                        nc.tensor.transpose(
                            pb[:, jj * P:(jj + 1) * P],
                            a_t[:, j * P:(j + 1) * P],
                            ident[:],
                        )
                    nc.vector.tensor_scalar_mul(
                        at_tiles[q][:, j8 * 8:(j8 + 1) * 8, t2 * P:(t2 + 1) * P],
                        pb[:].rearrange("p (j c) -> p j c", c=P),
                        SA,
                    )
                attn_tiles[t] = None

            def m_chains(q):
                at_sb = at_tiles[q]
                for dc in range(ET):
                    ps = psA.tile([P, 1024], f32, tag="psA")
                    for jp in range(SJT // 2):
                        nc.tensor.matmul(
                            ps[:, 0:256],
                            lhsT=xn_sb[:, 2 * jp:2 * jp + 2, dc * P:(dc + 1) * P],
                            rhs=at_sb[:, 2 * jp:2 * jp + 2, :],
                            start=(jp == 0), stop=(jp == SJT // 2 - 1),
                            perf_mode=DR,
                        )
                    nc.vector.tensor_scalar_mul(
                        m_sb[:, dc, q * 256:(q + 1) * 256], ps[:, 0:256],
                        SM / (SX * SA),
                    )

            scores_softmax(0)
            scores_softmax(1)
            for t in range(2, SIT):
                scores_softmax(t)
                transposes(t - 2)
                if t % 2 == 1:
                    m_chains((t - 2) // 2)
            transposes(SIT - 2)
            transposes(SIT - 1)
            m_chains(3)

            if h == H - 1:
                attn_ctx.close()

            # ---- head^T = Wv^T @ M : [e, si]
            ht_sb = ht_pool.tile([P, ET, SI], fp8, tag="ht")
            for eb in range(ET):
                wv_eb = wqkv_pool.tile([P, ET, P], fp8, tag="wqkv")
                nc.sync.dma_start(out=wv_eb[:], in_=A["wvb"][h, eb])
                ps = psA.tile([P, 1024], f32, tag="psA")
                for nb in range(2):
                    for kp in range(ET // 2):
                        nc.tensor.matmul(
                            ps[:, nb * 512:(nb + 1) * 512],
                            lhsT=wv_eb[:, 2 * kp:2 * kp + 2, :],
                            rhs=m_sb[:, 2 * kp:2 * kp + 2, nb * 512:(nb + 1) * 512],
                            start=(kp == 0), stop=(kp == ET // 2 - 1),
                            perf_mode=DR,
                        )
                nc.scalar.mul(ht_sb[:, eb, :], ps[:], SH / (SW * SM))

            w1_h = w1_pool.tile([P, ET, D], fp8, tag="w1", name=f"w1_{h}")
            nc.sync.dma_start(out=w1_h[:], in_=A["w1"][h])
            if h < H - 1:
                # prefetch the next head's first K weights so its K chain
                # doesn't stall on the DMA at the head boundary
                wk_next = wqkv_pool.tile([P, ET, P], fp8, tag="wqkv",
                                         name=f"wk_pre{h + 1}")
                nc.sync.dma_start(out=wk_next[:], in_=A["wkb"][h + 1, 0])

            if h == H - 1:
                mid_ctx.close()

            # ---- proj += head_h @ W1_h (head 3's chains interleave with LN)
            def w1_chain(t, ht_sb=ht_sb, w1_h=w1_h, h=h):
                ps = psA.tile([P, 1024], f32, tag="psA")
                for nb in range(2):
                    for ep in range(ET // 2):
                        nc.tensor.matmul(
                            ps[:, nb * 512:(nb + 1) * 512],
                            lhsT=ht_sb[:, 2 * ep:2 * ep + 2, t * P:(t + 1) * P],
                            rhs=w1_h[:, 2 * ep:2 * ep + 2, nb * 512:(nb + 1) * 512],
                            start=(ep == 0), stop=(ep == ET // 2 - 1),
                            perf_mode=DR,
                        )
                if h == 0:
                    nc.scalar.mul(proj_sb[:, t, :], ps[:], 1.0 / (SH * SW1))
                else:
                    nc.vector.scalar_tensor_tensor(
                        out=proj_sb[:, t, :], in0=ps[:], scalar=1.0 / (SH * SW1),
                        in1=proj_sb[:, t, :], op0=Alu.mult, op1=Alu.add,
                    )

            if h < H - 1:
                for t in range(SIT):
                    w1_chain(t)
            else:
                last_w1_chain = w1_chain

        # ================= LN1 -> FFN2 -> LN2, fully per-si-tile =================
        with ExitStack() as lctx:
            lnp = lctx.enter_context(tc.tile_pool(name="lnp", bufs=1))
            xr_pool = lctx.enter_context(tc.tile_pool(name="xr", bufs=3))
            u_pool = lctx.enter_context(tc.tile_pool(name="up", bufs=3))
            sq_pool = lctx.enter_context(tc.tile_pool(name="sq", bufs=2))
            ybf_pool = lctx.enter_context(tc.tile_pool(name="ybf", bufs=4))
            yt_pool = lctx.enter_context(tc.tile_pool(name="yt", bufs=3))
            w2_pool = lctx.enter_context(tc.tile_pool(name="w2", bufs=8))
            st_pool = lctx.enter_context(tc.tile_pool(name="st", bufs=8))
            ot_pool = lctx.enter_context(tc.tile_pool(name="ot", bufs=3))

            if not trivial_gbe:
                gbe_sb = lnp.tile([P, 4, D], f32, tag="gbe")
                gbe_bc = bass.AP(
                    tensor=A["gbe"].tensor, offset=A["gbe"].offset,
                    ap=[[0, P], A["gbe"].ap[0], A["gbe"].ap[1]],
                )
                nc.gpsimd.dma_start(out=gbe_sb[:], in_=gbe_bc)
            xr_tiles = []
            for t in range(SIT):
                xr = xr_pool.tile([P, D], f32, tag="xr", name=f"xr{t}")
                nc.sync.dma_start(out=xr[:], in_=A["xres"][t * P:(t + 1) * P, :])
                xr_tiles.append(xr)

            w2_sb = lnp.tile([P, ET, D], bf16, tag="w2")
            nc.sync.dma_start(out=w2_sb[:], in_=A["w2"][:])

            def ln_stats(src, rsum):
                """-> (mu, rstd) [P,1] tiles from src [P,D] + its row-sum."""
                sq = sq_pool.tile([P, D], f32, tag="sq")
                sumsq = st_pool.tile([P, 1], f32, tag="sumsq")
                nc.scalar.activation(out=sq[:], in_=src, func=Act.Square,
                                     accum_out=sumsq[:])
                mu = st_pool.tile([P, 1], f32, tag="mu")
                nc.scalar.mul(mu[:], rsum, 1.0 / D)
                # (rsum*mu - sumsq) = -D*var;  std = sqrt(-1/D * that + eps)
                nv = st_pool.tile([P, 1], f32, tag="nv")
                nc.vector.scalar_tensor_tensor(
                    out=nv[:], in0=rsum, scalar=mu[:], in1=sumsq[:],
                    op0=Alu.mult, op1=Alu.subtract,
                )
                rstd = st_pool.tile([P, 1], f32, tag="rstd")
                nc.scalar.activation(out=rstd[:], in_=nv[:], func=Act.Sqrt,
                                     scale=-1.0 / D, bias=eps_sb[:])
                nc.vector.reciprocal(rstd[:], rstd[:])
                return mu, rstd

            # Software-pipelined tail with a 2-tile skew: engines execute their
            # queues in emission order, so tile t's stage-C ops are emitted
            # after tile t+2's stage-A ops — otherwise each tile's ~11us
            # serial LN latency fully serializes the tail.
            y_tiles = [None] * SIT
            yt_tiles = [None] * SIT

            def stage_a(t):
                """W1(t+2) + u1 -> LN1 stats -> y (bf16) -> y^T blocks."""
                if t + 2 < SIT:
                    last_w1_chain(t + 2)
                u1 = u_pool.tile([P, D], f32, tag="u")
                rs1 = st_pool.tile([P, 1], f32, tag="rs")
                nc.vector.scalar_tensor_tensor(
                    out=u1[:], in0=xr_tiles[t][:], scalar=1.0,
                    in1=proj_sb[:, t, :], op0=Alu.mult, op1=Alu.add,
                    accum_out=rs1[:],
                )
                mu1, rstd1 = ln_stats(u1[:], rs1[:])
                yb = ybf_pool.tile([P, D], bf16, tag="ybf")
                y_tiles[t] = yb
                nc.vector.tensor_scalar(
                    yb[:], u1[:], scalar1=mu1[:], scalar2=rstd1[:],
                    op0=Alu.subtract, op1=Alu.mult,
                )
                if not trivial_gbe:
                    nc.gpsimd.tensor_mul(yb[:], yb[:], gbe_sb[:, 0, :])
                    nc.gpsimd.tensor_add(yb[:], yb[:], gbe_sb[:, 1, :])
                yt_tile = yt_pool.tile([P, ET, P], bf16, tag="yt")
                yt_tiles[t] = yt_tile
                pb = psB.tile([P, 1024], bf16, tag="psB")
                for fb in range(ET):
                    nc.tensor.transpose(
                        pb[:, fb * P:(fb + 1) * P], yb[:, fb * P:(fb + 1) * P], ident[:]
                    )
                nc.vector.tensor_copy(
                    yt_tile[:], pb[:].rearrange("p (f c) -> p f c", c=P)
                )

            def stage_c(t):
                """z-chain -> u2 -> LN2 -> out DMA."""
                yt_tile = yt_tiles[t]
                ps = psA.tile([P, 1024], f32, tag="psA")
                for nb in range(2):
                    for kc in range(ET):
                        nc.tensor.matmul(
                            ps[:, nb * 512:(nb + 1) * 512],
                            lhsT=yt_tile[:, kc, :],
                            rhs=w2_sb[:, kc, nb * 512:(nb + 1) * 512],
                            start=(kc == 0), stop=False,
                        )
                    nc.tensor.matmul(
                        ps[:, nb * 512:(nb + 1) * 512],
                        lhsT=ones_sb[:, :],
                        rhs=buv_sb[:, nb * 512:(nb + 1) * 512],
                        start=False, stop=True,
                    )
                u2 = u_pool.tile([P, 1024], f32, tag="u")
                rs2 = st_pool.tile([P, 1], f32, tag="rs")
                nc.vector.scalar_tensor_tensor(
                    out=u2[:], in0=y_tiles[t][:], scalar=1.0,
                    in1=ps[:], op0=Alu.mult, op1=Alu.add,
                    accum_out=rs2[:],
                )
                mu2, rstd2 = ln_stats(u2[:], rs2[:])
                ot = ot_pool.tile([P, D], f32, tag="ot")
                nc.vector.tensor_scalar(
                    ot[:], u2[:], scalar1=mu2[:], scalar2=rstd2[:],
                    op0=Alu.subtract, op1=Alu.mult,
                )
                if not trivial_gbe:
                    nc.gpsimd.tensor_mul(ot[:], ot[:], gbe_sb[:, 2, :])
                    nc.gpsimd.tensor_add(ot[:], ot[:], gbe_sb[:, 3, :])
                nc.sync.dma_start(out=A["out"][t * P:(t + 1) * P, :], in_=ot[:])

            last_w1_chain(0)
            last_w1_chain(1)
            for i in range(SIT + 2):
                if i >= 2:
                    stage_c(i - 2)
                if i < SIT:
                    stage_a(i)

        tail_ctx.close()


def _build(trivial_gbe):
    import concourse.bass as bass
    import concourse.mybir as mybir
    import concourse.tile as tile
    from concourse import bacc

    f32 = mybir.dt.float32
    bf16 = mybir.dt.bfloat16
    fp8 = mybir.dt.float8e4

    nc = bacc.Bacc("TRN2", target_bir_lowering=False, debug=False, num_devices=8)
    A = {}

    def din(name, shape, dt):
        A[name] = nc.dram_tensor(name, shape, dt, kind="ExternalInput").ap()

    din("xt", [D, S], fp8)
    din("xn", [S, D], fp8)
    din("xres", [SI, D], f32)
    din("wqb", [H, ET, P, ET, P], fp8)
    din("wkb", [H, ET, P, ET, P], fp8)
    din("wvb", [H, ET, P, ET, P], fp8)
    din("w1", [H, P, ET, D], fp8)
    din("w2", [P, ET, D], bf16)
    din("bqr", [P, H * ET], f32)
    din("buv", [1, D], bf16)
    if not trivial_gbe:
        din("gbe", [4, D], f32)
    A["out"] = nc.dram_tensor("out", [SI, D], f32, kind="ExternalOutput").ap()

    with tile.TileContext(nc) as tc:
        _emit(nc, tc, A, trivial_gbe)
    nc.compile()
    return nc


def _get_nc(trivial_gbe=True):
    key = ("nc", trivial_gbe)
    if key not in _CACHE:
        _CACHE[key] = _build(trivial_gbe)
    return _CACHE[key]


def _prep_inputs(inputs):
    x = np.ascontiguousarray(inputs["embedding_matrix"], dtype=np.float32)
    Wq = np.asarray(inputs["Wq"], np.float32)
    bq = np.asarray(inputs["bq"], np.float32)
    Wv = np.asarray(inputs["Wv"], np.float32)
    bv = np.asarray(inputs["bv"], np.float32)
    Wk = np.asarray(inputs["Wk"], np.float32)
    W1 = np.asarray(inputs["W1"], np.float32)
    b1 = np.asarray(inputs["b1"], np.float32)
    W2 = np.asarray(inputs["W2"], np.float32)
    b2 = np.asarray(inputs["b2"], np.float32)
    g1 = np.asarray(inputs["g1"], np.float32)
    be1 = np.asarray(inputs["be1"], np.float32)
    g2 = np.asarray(inputs["g2"], np.float32)
    be2 = np.asarray(inputs["be2"], np.float32)

    trivial = (
        np.array_equal(g1, np.ones(D, np.float32))
        and np.array_equal(g2, np.ones(D, np.float32))
        and np.array_equal(be1, np.zeros(D, np.float32))
        and np.array_equal(be2, np.zeros(D, np.float32))
    )

    def pack_w(W):  # [H, D, D] -> [H, ET, P(row-in-block), ET(kc), P] lhsT blocks
        return np.ascontiguousarray(
            (W * SW).reshape(H, ET, P, ET, P).transpose(0, 3, 2, 1, 4).astype(F8)
        )

    wqb = pack_w(Wq)
    wkb = pack_w(Wk)
    wvb = pack_w(Wv)
    # W1 [H*D, D] -> [H, P(p), ET(eb), D] fp8 lhsT-pair layout for the proj chain
    w1b = np.ascontiguousarray(
        (W1 * SW1).reshape(H, ET, P, D).transpose(0, 2, 1, 3).astype(F8)
    )
    # W2 [D, D] -> [P(p), ET(kc), D] bf16 lhsT layout for the z-chain
    w2b = np.ascontiguousarray(
        W2.reshape(ET, P, D).transpose(1, 0, 2).astype(BF)
    )
    # bq rearranged so bias for (h, e-block c) is column h*ET+c: [P, H*ET], x SQK
    bqr = np.ascontiguousarray(
        (bq * SQK).reshape(H, ET, P).transpose(2, 0, 1).reshape(P, H * ET)
    )
    cvec = (b1 + sum(bv[h] @ W1[h * D:(h + 1) * D] for h in range(H)))
    buv = np.ascontiguousarray(b2.reshape(1, D).astype(BF))

    shared = {
        "wqb": wqb, "wkb": wkb, "wvb": wvb, "w1": w1b, "w2": w2b,
        "bqr": bqr, "buv": buv,
    }
    if not trivial:
        shared["gbe"] = np.ascontiguousarray(np.stack([g1, be1, g2, be2]))
    in_maps = []
    for core in range(8):
        b, half = core // 2, core % 2
        own = x[b, half * SI:(half + 1) * SI]
        other = x[b, (1 - half) * SI:(2 - half) * SI]
        xperm = np.concatenate([own, other], axis=0)
        m = dict(shared)
        m["xn"] = np.ascontiguousarray((xperm * SX).astype(F8))
        m["xt"] = np.ascontiguousarray((xperm.T * SX).astype(F8))
        m["xres"] = np.ascontiguousarray(own + cvec[None, :])
        in_maps.append(m)
    return trivial, in_maps


def kernel(**inputs):
    from concourse.bass_utils import run_bass_kernel_spmd

    trivial, in_maps = _prep_inputs(inputs)
    nc = _get_nc(trivial)
    res = run_bass_kernel_spmd(nc, in_maps, core_ids=list(range(8)))
    out = np.empty((4, S, D), np.float32)
    for core in range(8):
        b, half = core // 2, core % 2
        out[b, half * SI:(half + 1) * SI] = res.results[core]["out"]
    return out
